# revision 9
# baseline (speedup 1.0000x reference)
"""DDALoss Trainium2 kernel v4 (8 NeuronCores, data-parallel over batch).

Math (algebraically identical to the reference up to tiny fp8/trunc noise):
  g[n,c]     = 2*feat[n]@centers[c] - ||centers[c]||^2          (logits shifted
               by the row-constant ||feat[n]||^2, which cancels in softmax)
  lse[n]     = log(sum_c exp(g[n,c]))
  glab[n]    = g[n, label[n]]
  nll_sum    = sum_n (lse[n] - glab[n])
  S1         = sum(feat^2)
  centerloss = (S1 - sum_n glab[n]) / (2N)
  ddaloss    = nll_sum / (2N^2)
  loss       = LAMB*centerloss + GAMMA*ddaloss

v4 design (per core: 512 rows, 10240 padded classes, 20 chunks of 512):
  - csq on the PE: per 128-class tile, a 2-pass fp8-DoubleRow Gram matmul
    (ct-tile as both weights and moving) puts CS^2*csq on the psum diagonal
    (128 cols/pass vs 512 for a bias pass -> half the PE cost of the old
    K=1 bias matmuls, and no 10.5MB cnat load / DVE square pass at all).
  - diagonal extraction: DVE copy psum->bf16 with scale -FS/(2CS), bounce
    [128,512] to DRAM, read back with a stride-513 AP -> [128,4] diag.
  - bias inject: diag split hi/lo fp8 (DVE casts), written to DRAM in class
    order, then patched into partitions 126/127 of each ct chunk's k3 block.
    ft rows 126/127 of k3 are 1.0, so the second matmul pass adds
    -(FS*CS/2)*csq into psum for free (feat dims 510/511 are dropped from
    the softmax cross term only; effect ~1e-5 relative).
  - main loop: per (nt, group of 3 chunks): 2 DR passes per chunk into a
    [128,1536] psum, then one ACT exp (scale 2/(FS*CS)) with accum_out.
  - label path (indirect gather + bf16 TTR dots) and finals as before.
"""

import sys

sys.path.insert(0, "/opt/trn_rl_repo")

import numpy as np
import ml_dtypes

from contextlib import ExitStack

import concourse.bass as bass
import concourse.bacc as bacc
import concourse.tile as tile
from concourse import mybir

# Problem constants (hardcoded per harness contract)
N = 4096
D = 512
C = 10000
CP = 10240  # classes padded to 128*80
NCORES = 8
NPC = N // NCORES  # 512 rows per core
NT = NPC // 128  # 4 partition tiles per core
NCH = 20  # class chunks of 512
CSZ = 512
GRPS = [3, 3, 3, 3, 3, 3, 2]  # ACT groups, in chunks
NGRP = len(GRPS)
GRP_OFF = [sum(GRPS[:i]) for i in range(NGRP)]
assert sum(GRPS) == NCH

LAMB = 0.01
GAMMA = 3.0

BF16 = mybir.dt.bfloat16
FP8 = mybir.dt.float8e4
F32 = mybir.dt.float32
I32 = mybir.dt.int32

# fp8 scaling: feat*FS and centers*CS on host keep e4m3 values in the normal
# range; psum holds FS*CS*cross + bias rows, ACT's exp scale of 2/(FS*CS)
# restores exp(2*cross - csq).
FS = 8.0
CS = 16.0
BSC = -FS / (2.0 * CS)  # psum-units bias = BSC * (CS^2 * csq) = -(FS*CS/2)*csq

_CACHE = {}


def _ttr(nc, out, in0, in1, accum_out, init, scale=1.0):
    """accum_out = init + sum_free(in0 * in1 * scale); out = elementwise scratch."""
    from concourse.dve_ops import TENSOR_TENSOR_REDUCE

    nc.vector._custom_dve(
        TENSOR_TENSOR_REDUCE,
        out=out,
        in0=in0,
        in1=in1,
        s0=init,
        s1=scale,
        accum_out=accum_out,
    )


def _build():
    nc = bacc.Bacc("TRN2", target_bir_lowering=False, debug=False)

    # Per-core external inputs
    ft_d = nc.dram_tensor("ftt", [128, 4, NPC], FP8, kind="ExternalInput")
    ct_d = nc.dram_tensor("ct", [128, 4 * CP], FP8, kind="ExternalInput")  # chunk-major
    fnat = nc.dram_tensor("fnat", [NPC, D], BF16, kind="ExternalInput")
    lab = nc.dram_tensor("lab", [NPC, 1], I32, kind="ExternalInput")
    crs = nc.dram_tensor("crs", [CP, D], BF16, kind="ExternalInput")  # gather rows
    padrow = nc.dram_tensor("padrow", [1, CP - C], FP8, kind="ExternalInput")
    out = nc.dram_tensor("out", [1, 3], F32, kind="ExternalOutput")
    # internal scratch
    gram_dram = nc.dram_tensor("gram_scr", [NCH * 128, CSZ], BF16, kind="Internal")
    csq2 = nc.dram_tensor("csq2_scr", [2, CP], FP8, kind="Internal")

    with tile.TileContext(nc) as tc, ExitStack() as ctx:
        const = ctx.enter_context(tc.tile_pool(name="const", bufs=1))
        small = ctx.enter_context(tc.tile_pool(name="small", bufs=2))
        gbfp = ctx.enter_context(tc.tile_pool(name="gbfp", bufs=3))
        dgp = ctx.enter_context(tc.tile_pool(name="dgp", bufs=3))
        expp = ctx.enter_context(tc.tile_pool(name="expp", bufs=2))
        scrp = ctx.enter_context(tc.tile_pool(name="scrp", bufs=2))
        ps_g = ctx.enter_context(tc.tile_pool(name="ps_g", bufs=2, space="PSUM"))
        ps_gram = ctx.enter_context(tc.tile_pool(name="ps_gram", bufs=2, space="PSUM"))

        # ---- constants / persistent tiles ----
        ones_f = const.tile([128, 1], F32)
        nc.vector.memset(ones_f, 1.0)

        ft = const.tile([128, 4, NPC], FP8, tag="ft")
        nc.sync.dma_start(out=ft, in_=ft_d.ap())

        ct_all = const.tile([128, NCH, 4, CSZ], FP8, tag="ct_all")
        accg = const.tile([128, NT * NGRP], F32, tag="accg")
        cl4 = const.tile([128, NT], F32, tag="cl4")
        cq4 = const.tile([128, NT], F32, tag="cq4")
        fsq4 = const.tile([128, NT], F32, tag="fsq4")
        fin3 = const.tile([128, 3], F32, tag="fin3")

        ct_ap = ct_d.ap().rearrange("p (ch k c) -> p ch k c", k=4, c=CSZ)

        ct_dmas = {}

        def ct_dma(ci):
            ct_dmas[ci] = nc.sync.dma_start(
                out=ct_all[:, ci, :, :], in_=ct_ap[:, ci, :, :]
            )

        def gram_chain(ci):
            # Gram diag: psum[a, 128t+b] = sum_d (CS c[d,a+])(CS c[d,b+])
            gp = ps_gram.tile([128, CSZ], F32, tag="gp")
            for t in range(4):
                cb = slice(128 * t, 128 * (t + 1))
                nc.tensor.matmul(
                    out=gp[:, cb],
                    lhsT=ct_all[:, ci, 0:2, cb],
                    rhs=ct_all[:, ci, 0:2, cb],
                    start=True,
                    stop=False,
                    perf_mode=mybir.MatmulPerfMode.DoubleRow,
                )
                nc.tensor.matmul(
                    out=gp[:, cb],
                    lhsT=ct_all[:, ci, 2:4, cb],
                    rhs=ct_all[:, ci, 2:4, cb],
                    start=False,
                    stop=True,
                    perf_mode=mybir.MatmulPerfMode.DoubleRow,
                )
            gb = gbfp.tile([128, CSZ], BF16, tag="gb")
            nc.vector.tensor_scalar_mul(gb, gp, BSC)  # -> -(FS/2CS)*CS^2*csq, bf16
            nc.sync.dma_start(
                out=bass.AP(tensor=gram_dram, offset=65536 * ci, ap=[[512, 128], [1, 512]]),
                in_=gb,
            )
            dh = dgp.tile([128, 4], BF16, tag="dh")
            nc.sync.dma_start(
                out=dh,
                in_=bass.AP(tensor=gram_dram, offset=65536 * ci, ap=[[513, 128], [128, 4]]),
            )
            h8 = dgp.tile([128, 4], FP8, tag="h8")
            nc.vector.tensor_copy(h8, dh)
            hb = dgp.tile([128, 4], BF16, tag="hb")
            nc.vector.tensor_copy(hb, h8)
            lo = dgp.tile([128, 4], BF16, tag="lo")
            nc.vector.tensor_sub(lo, dh, hb)
            l8 = dgp.tile([128, 4], FP8, tag="l8")
            nc.vector.tensor_copy(l8, lo)
            nc.sync.dma_start(
                out=bass.AP(tensor=csq2, offset=512 * ci, ap=[[1, 128], [128, 4]]),
                in_=h8,
            )
            nc.sync.dma_start(
                out=bass.AP(tensor=csq2, offset=CP + 512 * ci, ap=[[1, 128], [128, 4]]),
                in_=l8,
            )
            if ci == NCH - 1:
                # pad classes: force hi row to -448 (-> exp(-7) ~ 0 per pad)
                nc.sync.dma_start(
                    out=bass.AP(tensor=csq2, offset=C, ap=[[0, 1], [1, CP - C]]),
                    in_=padrow.ap(),
                )
            # patch hi/lo rows into partitions 126/127 of this chunk's k3 block
            nc.sync.dma_start(
                out=ct_all[126:128, ci, 3, :],
                in_=bass.AP(tensor=csq2, offset=512 * ci, ap=[[CP, 2], [1, 512]]),
            )

        # ---- prologue: prefetch + gram pipeline head ----
        for ci in range(3):
            ct_dma(ci)
        for ci in range(3):
            ct_dma(ci + 3)
            gram_chain(ci)

        dma_cursor = 6
        gram_cursor = 3

        # ---- main loop ----
        for nt in range(NT):
            for g in range(NGRP):
                w = GRPS[g] * CSZ
                gps = ps_g.tile([128, GRPS[0] * CSZ], F32, tag="gps")
                for j in range(GRPS[g]):
                    ci = GRP_OFF[g] + j
                    if nt == 0:
                        while gram_cursor <= min(ci + 3, NCH - 1):
                            if dma_cursor < NCH:
                                ct_dma(dma_cursor)
                                dma_cursor += 1
                            gram_chain(gram_cursor)
                            gram_cursor += 1
                    cb = slice(CSZ * j, CSZ * (j + 1))
                    nc.tensor.matmul(
                        out=gps[:, cb],
                        lhsT=ft[:, 0:2, nt * 128 : (nt + 1) * 128],
                        rhs=ct_all[:, ci, 0:2, :],
                        start=True,
                        stop=False,
                        perf_mode=mybir.MatmulPerfMode.DoubleRow,
                    )
                    nc.tensor.matmul(
                        out=gps[:, cb],
                        lhsT=ft[:, 2:4, nt * 128 : (nt + 1) * 128],
                        rhs=ct_all[:, ci, 2:4, :],
                        start=False,
                        stop=True,
                        perf_mode=mybir.MatmulPerfMode.DoubleRow,
                    )
                se = expp.tile([128, GRPS[0] * CSZ], BF16, tag="se")
                col = nt * NGRP + g
                nc.scalar.activation(
                    se[:, :w],
                    gps[:, :w],
                    mybir.ActivationFunctionType.Exp,
                    scale=2.0 / (FS * CS),
                    accum_out=accg[:, col : col + 1],
                )

            if nt == 1:
                # ---- label path (independent; fills engine gaps late) ----
                for lt in range(NT):
                    labt = small.tile([128, 1], I32, tag="labt")
                    d1 = nc.sync.dma_start(
                        out=labt, in_=lab.ap()[lt * 128 : (lt + 1) * 128, :]
                    )
                    tile.add_dep_helper(
                        d1.ins, ct_dmas[NCH - 1].ins, True, "defer label path"
                    )
                    crows = small.tile([128, D], BF16, tag="crows")
                    nc.gpsimd.indirect_dma_start(
                        out=crows,
                        out_offset=None,
                        in_=crs.ap(),
                        in_offset=bass.IndirectOffsetOnAxis(ap=labt[:, :1], axis=0),
                    )
                    fnt = small.tile([128, D], BF16, tag="fnt")
                    d2 = nc.sync.dma_start(
                        out=fnt, in_=fnat.ap()[lt * 128 : (lt + 1) * 128, :]
                    )
                    tile.add_dep_helper(
                        d2.ins, ct_dmas[NCH - 1].ins, True, "defer label path"
                    )
                    scr1 = scrp.tile([128, D], BF16, tag="lab_scr")
                    _ttr(nc, scr1, fnt, crows, cl4[:, lt : lt + 1], 0.0)
                    scr2 = scrp.tile([128, D], BF16, tag="lab_scr")
                    _ttr(nc, scr2, crows, crows, cq4[:, lt : lt + 1], 0.0)
                    scr3 = scrp.tile([128, D], BF16, tag="lab_scr")
                    _ttr(nc, scr3, fnt, fnt, fsq4[:, lt : lt + 1], 0.0)

        # ---- finals ----
        sumexp4 = small.tile([128, NT], F32, tag="sumexp4")
        nc.vector.reduce_sum(
            sumexp4,
            accg[:, :].rearrange("p (nt g) -> p nt g", g=NGRP),
            axis=mybir.AxisListType.X,
        )
        lse4 = small.tile([128, NT], F32, tag="lse4")
        nc.scalar.activation(lse4, sumexp4, mybir.ActivationFunctionType.Ln)
        glab4 = small.tile([128, NT], F32, tag="glab4")
        nc.vector.tensor_scalar_mul(glab4, cl4, 2.0)
        nc.vector.tensor_sub(glab4, glab4, cq4)
        nld4 = small.tile([128, NT], F32, tag="nld4")
        nc.vector.tensor_sub(nld4, lse4, glab4)
        nc.vector.reduce_sum(fin3[:, 0:1], nld4, axis=mybir.AxisListType.X)
        nc.vector.reduce_sum(fin3[:, 1:2], glab4, axis=mybir.AxisListType.X)
        nc.vector.reduce_sum(fin3[:, 2:3], fsq4, axis=mybir.AxisListType.X)
        fin_ps = ps_gram.tile([128, CSZ], F32, tag="gp")
        nc.tensor.matmul(
            out=fin_ps[0:1, 0:3], lhsT=ones_f, rhs=fin3, start=True, stop=True
        )
        out_sb = small.tile([1, 3], F32, tag="out_sb")
        nc.scalar.copy(out_sb, fin_ps[0:1, 0:3])
        nc.sync.dma_start(out=out.ap(), in_=out_sb)

    nc.compile()
    return nc


def _get_nc():
    if "nc" not in _CACHE:
        _CACHE["nc"] = _build()
    return _CACHE["nc"]


def make_in_maps(feat, label, centers):
    feat = np.ascontiguousarray(np.asarray(feat, dtype=np.float32))
    centers = np.ascontiguousarray(np.asarray(centers, dtype=np.float32))
    label = np.ascontiguousarray(np.asarray(label).astype(np.int32).reshape(N, 1))

    bf = ml_dtypes.bfloat16
    f8 = ml_dtypes.float8_e4m3

    # ft pack: [128, 4, N]; k3 = dims 384..509 + two ones rows
    ft_pack = np.zeros((128, 4, N), dtype=f8)
    fT = (feat.T * FS).astype(f8)  # [D, N]
    for kb in range(3):
        ft_pack[:, kb, :] = fT[128 * kb : 128 * (kb + 1), :]
    ft_pack[0:126, 3, :] = fT[384:510, :]
    ft_pack[126:128, 3, :] = 1.0

    # ct pack: same dim map, chunk-major [128, 4*CP]
    ct_pack = np.zeros((128, 4, CP), dtype=f8)
    cT = (centers.T * CS).astype(f8)  # [D, C]
    for kb in range(3):
        ct_pack[:, kb, :C] = cT[128 * kb : 128 * (kb + 1), :]
    ct_pack[0:126, 3, :C] = cT[384:510, :]
    ct_cm = np.ascontiguousarray(
        ct_pack.reshape(128, 4, NCH, CSZ).transpose(0, 2, 1, 3).reshape(128, 4 * CP)
    )

    crs_pad = np.zeros((CP, D), dtype=bf)
    crs_pad[:C, :] = centers.astype(bf)
    fnat_bf = feat.astype(bf)
    padrow = np.full((1, CP - C), -240.0, dtype=f8)

    in_maps = []
    for i in range(NCORES):
        sl = slice(i * NPC, (i + 1) * NPC)
        in_maps.append(
            {
                "ftt": np.ascontiguousarray(ft_pack[:, :, sl]),
                "fnat": np.ascontiguousarray(fnat_bf[sl]),
                "lab": np.ascontiguousarray(label[sl]),
                "ct": ct_cm,
                "crs": crs_pad,
                "padrow": padrow,
            }
        )
    return in_maps


def combine(parts):
    nll_sum, glab_sum, s1 = np.asarray(parts, dtype=np.float64).sum(axis=0)
    centerloss = (s1 - glab_sum) / (2.0 * N)
    ddaloss = nll_sum / (2.0 * N * N)
    loss = LAMB * centerloss + GAMMA * ddaloss
    return loss, centerloss, ddaloss


def kernel(feat, label, centers):
    from concourse.bass_utils import run_bass_kernel_spmd

    in_maps = make_in_maps(feat, label, centers)
    nc = _get_nc()
    res = run_bass_kernel_spmd(nc, in_maps, core_ids=list(range(NCORES)))
    parts = [r["out"].reshape(3) for r in res.results]
    loss, centerloss, ddaloss = combine(parts)
    return (
        np.float32(loss),
        np.float32(centerloss),
        np.float32(ddaloss),
    )


# revision 17
# speedup vs baseline: 4.8734x; 4.8734x over previous
"""DDALoss Trainium2 kernel v4 (8 NeuronCores, data-parallel over batch).

Math (algebraically identical to the reference up to tiny fp8/trunc noise):
  g[n,c]     = 2*feat[n]@centers[c] - ||centers[c]||^2          (logits shifted
               by the row-constant ||feat[n]||^2, which cancels in softmax)
  lse[n]     = log(sum_c exp(g[n,c]))
  glab[n]    = g[n, label[n]]
  nll_sum    = sum_n (lse[n] - glab[n])
  S1         = sum(feat^2)
  centerloss = (S1 - sum_n glab[n]) / (2N)
  ddaloss    = nll_sum / (2N^2)
  loss       = LAMB*centerloss + GAMMA*ddaloss

v4 design (per core: 512 rows, 10240 padded classes, 20 chunks of 512):
  - csq on the PE: per 128-class tile, a 2-pass fp8-DoubleRow Gram matmul
    (ct-tile as both weights and moving) puts CS^2*csq on the psum diagonal
    (128 cols/pass vs 512 for a bias pass -> half the PE cost of the old
    K=1 bias matmuls, and no 10.5MB cnat load / DVE square pass at all).
  - diagonal extraction: DVE copy psum->bf16 with scale -FS/(2CS), bounce
    [128,512] to DRAM, read back with a stride-513 AP -> [128,4] diag.
  - bias inject: diag split hi/lo fp8 (DVE casts), written to DRAM in class
    order, then patched into partitions 126/127 of each ct chunk's k3 block.
    ft rows 126/127 of k3 are 1.0, so the second matmul pass adds
    -(FS*CS/2)*csq into psum for free (feat dims 510/511 are dropped from
    the softmax cross term only; effect ~1e-5 relative).
  - main loop: per (nt, group of 3 chunks): 2 DR passes per chunk into a
    [128,1536] psum, then one ACT exp (scale 2/(FS*CS)) with accum_out.
  - label path (indirect gather + bf16 TTR dots) and finals as before.
"""

import sys

sys.path.insert(0, "/opt/trn_rl_repo")

import numpy as np
import ml_dtypes

from contextlib import ExitStack

import concourse.bass as bass
import concourse.bacc as bacc
import concourse.tile as tile
from concourse import mybir

# Problem constants (hardcoded per harness contract)
N = 4096
D = 512
C = 10000
CP = 10240  # classes padded to 128*80
NCORES = 8
NPC = N // NCORES  # 512 rows per core
NT = NPC // 128  # 4 partition tiles per core
NCH = 20  # class chunks of 512
CSZ = 512
GRPS = [3, 3, 3, 3, 3, 3, 2]  # ACT groups, in chunks
NGRP = len(GRPS)
GRP_OFF = [sum(GRPS[:i]) for i in range(NGRP)]
assert sum(GRPS) == NCH

LAMB = 0.01
GAMMA = 3.0

BF16 = mybir.dt.bfloat16
FP8 = mybir.dt.float8e4
F32 = mybir.dt.float32
I32 = mybir.dt.int32

# fp8 scaling: feat*FS and centers*CS on host keep e4m3 values in the normal
# range; psum holds FS*CS*cross + bias rows, ACT's exp scale of 2/(FS*CS)
# restores exp(2*cross - csq).
FS = 8.0
CS = 16.0
BSC = -FS / (2.0 * CS)  # psum-units bias = BSC * (CS^2 * csq) = -(FS*CS/2)*csq

_CACHE = {}


def _ttr(nc, out, in0, in1, accum_out, init, scale=1.0):
    """accum_out = init + sum_free(in0 * in1 * scale); out = elementwise scratch."""
    from concourse.dve_ops import TENSOR_TENSOR_REDUCE

    nc.vector._custom_dve(
        TENSOR_TENSOR_REDUCE,
        out=out,
        in0=in0,
        in1=in1,
        s0=init,
        s1=scale,
        accum_out=accum_out,
    )


def _build():
    nc = bacc.Bacc("TRN2", target_bir_lowering=False, debug=False)

    # Per-core external inputs
    ft_d = nc.dram_tensor("ftt", [128, 4, NPC], FP8, kind="ExternalInput")
    ct_d = nc.dram_tensor("ct", [128, 4 * CP], FP8, kind="ExternalInput")  # chunk-major
    fnat = nc.dram_tensor("fnat", [NPC, D], BF16, kind="ExternalInput")
    lab = nc.dram_tensor("lab", [NPC, 1], I32, kind="ExternalInput")
    crs = nc.dram_tensor("crs", [CP, D], BF16, kind="ExternalInput")  # gather rows
    padrow = nc.dram_tensor("padrow", [1, CP - C], FP8, kind="ExternalInput")
    out = nc.dram_tensor("out", [1, 3], F32, kind="ExternalOutput")
    # internal scratch
    csq2 = nc.dram_tensor("csq2_scr", [2, CP], FP8, kind="Internal")

    with tile.TileContext(nc) as tc, ExitStack() as ctx:
        const = ctx.enter_context(tc.tile_pool(name="const", bufs=1))
        small = ctx.enter_context(tc.tile_pool(name="small", bufs=2))
        gbfp = ctx.enter_context(tc.tile_pool(name="gbfp", bufs=3))
        dgp = ctx.enter_context(tc.tile_pool(name="dgp", bufs=6))
        expp = ctx.enter_context(tc.tile_pool(name="expp", bufs=2))
        scrp = ctx.enter_context(tc.tile_pool(name="scrp", bufs=2))
        ps_g = ctx.enter_context(tc.tile_pool(name="ps_g", bufs=2, space="PSUM"))
        ps_gram = ctx.enter_context(tc.tile_pool(name="ps_gram", bufs=2, space="PSUM"))

        # ---- constants / persistent tiles ----
        ones_f = const.tile([128, 1], F32)
        nc.vector.memset(ones_f, 1.0)
        ident = const.tile([128, 128], F32, tag="ident")
        from concourse.masks import make_identity

        make_identity(nc, ident)

        ft = const.tile([128, 4, NPC], FP8, tag="ft")
        nc.sync.dma_start(out=ft, in_=ft_d.ap())

        ct_all = const.tile([128, NCH, 4, CSZ], FP8, tag="ct_all")
        accg = const.tile([128, NT * NGRP], F32, tag="accg")
        cl4 = const.tile([128, NT], F32, tag="cl4")
        cq4 = const.tile([128, NT], F32, tag="cq4")
        fsq4 = const.tile([128, NT], F32, tag="fsq4")
        fin3 = const.tile([128, 3], F32, tag="fin3")

        ct_ap = ct_d.ap().rearrange("p (ch k c) -> p ch k c", k=4, c=CSZ)

        ct_dmas = {}

        def ct_dma(ci):
            ct_dmas[ci] = nc.sync.dma_start(
                out=ct_all[:, ci, :, :], in_=ct_ap[:, ci, :, :]
            )

        dhb_tiles = {}  # batch index -> [128, 16] diag tile (col = 4*(ci%4)+t)

        def chain_head(ci):
            # Gram: psum[a, 128t+b] = sum_d (CS c[d,128t+a])(CS c[d,128t+b]);
            # diag extracted by identity-masked TTR straight off PSUM.
            gp = ps_gram.tile([128, CSZ], F32, tag="gp")
            for t in range(4):
                cb = slice(128 * t, 128 * (t + 1))
                nc.tensor.matmul(
                    out=gp[:, cb],
                    lhsT=ct_all[:, ci, 0:2, cb],
                    rhs=ct_all[:, ci, 0:2, cb],
                    start=True,
                    stop=False,
                    perf_mode=mybir.MatmulPerfMode.DoubleRow,
                )
                nc.tensor.matmul(
                    out=gp[:, cb],
                    lhsT=ct_all[:, ci, 2:4, cb],
                    rhs=ct_all[:, ci, 2:4, cb],
                    start=False,
                    stop=True,
                    perf_mode=mybir.MatmulPerfMode.DoubleRow,
                )
            b = ci // 4
            if b not in dhb_tiles:
                dhb_tiles[b] = dgp.tile([128, 16], F32, tag="dhb", name=f"dhb{b}")
            dhb = dhb_tiles[b]
            for t in range(4):
                scr = gbfp.tile([128, 128], F32, tag="dscr")
                _ttr(
                    nc,
                    scr,
                    gp[:, 128 * t : 128 * (t + 1)],
                    ident,
                    dhb[:, 4 * (ci % 4) + t : 4 * (ci % 4) + t + 1],
                    0.0,
                    scale=BSC,
                )

        def tail_batch(b):
            # 4 chunks: transpose [128,16] -> [16,128]; row r=4c+t -> class
            # offset 128*r; fp8 hi/lo; contiguous DMAs; patch each chunk.
            dhb = dhb_tiles.pop(b)
            tp = ps_gram.tile([128, CSZ], F32, tag="gp")
            nc.tensor.transpose(out=tp[0:16, 0:128], in_=dhb, identity=ident)
            dtr = dgp.tile([16, 128], F32, tag="dtr")
            nc.vector.tensor_copy(dtr, tp[0:16, 0:128])
            h8 = dgp.tile([16, 128], FP8, tag="h8")
            nc.vector.tensor_copy(h8, dtr)
            hb = dgp.tile([16, 128], F32, tag="hb")
            nc.vector.tensor_copy(hb, h8)
            lo = dgp.tile([16, 128], F32, tag="lo")
            nc.vector.tensor_sub(lo, dtr, hb)
            l8 = dgp.tile([16, 128], FP8, tag="l8")
            nc.vector.tensor_copy(l8, lo)
            nc.sync.dma_start(
                out=bass.AP(tensor=csq2, offset=2048 * b, ap=[[128, 16], [1, 128]]),
                in_=h8,
            )
            nc.sync.dma_start(
                out=bass.AP(tensor=csq2, offset=CP + 2048 * b, ap=[[128, 16], [1, 128]]),
                in_=l8,
            )
            if b == 4:
                # pad classes: force hi row to -240 (-> exp(-3.75) ~ 0.02 per pad)
                nc.sync.dma_start(
                    out=bass.AP(tensor=csq2, offset=C, ap=[[0, 1], [1, CP - C]]),
                    in_=padrow.ap(),
                )
            for ci in range(4 * b, 4 * b + 4):
                nc.sync.dma_start(
                    out=ct_all[126:128, ci, 3, :],
                    in_=bass.AP(tensor=csq2, offset=512 * ci, ap=[[CP, 2], [1, 512]]),
                )

        # ---- prologue: prefetch + pipeline heads ----
        for ci in range(10):
            ct_dma(ci)
        dma_cursor = 10
        for ci in range(8):
            chain_head(ci)
        head_cursor = 8
        tail_batch(0)
        tail_cursor = 1

        def main_group(nt, g):
            w = GRPS[g] * CSZ
            gps = ps_g.tile([128, GRPS[0] * CSZ], F32, tag="gps")
            for j in range(GRPS[g]):
                ci = GRP_OFF[g] + j
                cb = slice(CSZ * j, CSZ * (j + 1))
                nc.tensor.matmul(
                    out=gps[:, cb],
                    lhsT=ft[:, 0:2, nt * 128 : (nt + 1) * 128],
                    rhs=ct_all[:, ci, 0:2, :],
                    start=True,
                    stop=False,
                    perf_mode=mybir.MatmulPerfMode.DoubleRow,
                )
                nc.tensor.matmul(
                    out=gps[:, cb],
                    lhsT=ft[:, 2:4, nt * 128 : (nt + 1) * 128],
                    rhs=ct_all[:, ci, 2:4, :],
                    start=False,
                    stop=True,
                    perf_mode=mybir.MatmulPerfMode.DoubleRow,
                )
            se = expp.tile([128, GRPS[0] * CSZ], BF16, tag="se")
            col = nt * NGRP + g
            nc.scalar.activation(
                se[:, :w],
                gps[:, :w],
                mybir.ActivationFunctionType.Exp,
                scale=2.0 / (FS * CS),
                accum_out=accg[:, col : col + 1],
            )

        # phase A: nt0/nt1 interleaved, heads/tails paced between groups
        for s in range(2 * NGRP):
            while head_cursor < min(8 + 2 * (s + 1), NCH):
                if dma_cursor < NCH:
                    ct_dma(dma_cursor)
                    dma_cursor += 1
                chain_head(head_cursor)
                head_cursor += 1
            while tail_cursor <= 4 and head_cursor >= 4 * (tail_cursor + 1):
                tail_batch(tail_cursor)
                tail_cursor += 1
            main_group(s % 2, s // 2)
        # phase B: nt2, nt3
        for nt in (2, 3):
            for g in range(NGRP):
                main_group(nt, g)

            if nt == 2:
                # ---- label path (independent; fills engine gaps late) ----
                for lt in range(NT):
                    labt = small.tile([128, 1], I32, tag="labt")
                    d1 = nc.sync.dma_start(
                        out=labt, in_=lab.ap()[lt * 128 : (lt + 1) * 128, :]
                    )
                    tile.add_dep_helper(
                        d1.ins, ct_dmas[NCH - 1].ins, True, "defer label path"
                    )
                    crows = small.tile([128, D], BF16, tag="crows")
                    nc.gpsimd.indirect_dma_start(
                        out=crows,
                        out_offset=None,
                        in_=crs.ap(),
                        in_offset=bass.IndirectOffsetOnAxis(ap=labt[:, :1], axis=0),
                    )
                    fnt = small.tile([128, D], BF16, tag="fnt")
                    d2 = nc.sync.dma_start(
                        out=fnt, in_=fnat.ap()[lt * 128 : (lt + 1) * 128, :]
                    )
                    tile.add_dep_helper(
                        d2.ins, ct_dmas[NCH - 1].ins, True, "defer label path"
                    )
                    scr1 = scrp.tile([128, D], BF16, tag="lab_scr")
                    _ttr(nc, scr1, fnt, crows, cl4[:, lt : lt + 1], 0.0)
                    scr2 = scrp.tile([128, D], BF16, tag="lab_scr")
                    _ttr(nc, scr2, crows, crows, cq4[:, lt : lt + 1], 0.0)
                    scr3 = scrp.tile([128, D], BF16, tag="lab_scr")
                    _ttr(nc, scr3, fnt, fnt, fsq4[:, lt : lt + 1], 0.0)

        # ---- finals ----
        sumexp4 = small.tile([128, NT], F32, tag="sumexp4")
        nc.vector.reduce_sum(
            sumexp4,
            accg[:, :].rearrange("p (nt g) -> p nt g", g=NGRP),
            axis=mybir.AxisListType.X,
        )
        lse4 = small.tile([128, NT], F32, tag="lse4")
        nc.scalar.activation(lse4, sumexp4, mybir.ActivationFunctionType.Ln)
        glab4 = small.tile([128, NT], F32, tag="glab4")
        nc.vector.tensor_scalar_mul(glab4, cl4, 2.0)
        nc.vector.tensor_sub(glab4, glab4, cq4)
        nld4 = small.tile([128, NT], F32, tag="nld4")
        nc.vector.tensor_sub(nld4, lse4, glab4)
        nc.vector.reduce_sum(fin3[:, 0:1], nld4, axis=mybir.AxisListType.X)
        nc.vector.reduce_sum(fin3[:, 1:2], glab4, axis=mybir.AxisListType.X)
        nc.vector.reduce_sum(fin3[:, 2:3], fsq4, axis=mybir.AxisListType.X)
        fin_ps = ps_gram.tile([128, CSZ], F32, tag="gp")
        nc.tensor.matmul(
            out=fin_ps[0:1, 0:3], lhsT=ones_f, rhs=fin3, start=True, stop=True
        )
        out_sb = small.tile([1, 3], F32, tag="out_sb")
        nc.scalar.copy(out_sb, fin_ps[0:1, 0:3])
        nc.sync.dma_start(out=out.ap(), in_=out_sb)

    nc.compile()
    return nc


def _get_nc():
    if "nc" not in _CACHE:
        _CACHE["nc"] = _build()
    return _CACHE["nc"]


def make_in_maps(feat, label, centers):
    feat = np.ascontiguousarray(np.asarray(feat, dtype=np.float32))
    centers = np.ascontiguousarray(np.asarray(centers, dtype=np.float32))
    label = np.ascontiguousarray(np.asarray(label).astype(np.int32).reshape(N, 1))

    bf = ml_dtypes.bfloat16
    f8 = ml_dtypes.float8_e4m3

    # ft pack: [128, 4, N]; k3 = dims 384..509 + two ones rows
    ft_pack = np.zeros((128, 4, N), dtype=f8)
    fT = (feat.T * FS).astype(f8)  # [D, N]
    for kb in range(3):
        ft_pack[:, kb, :] = fT[128 * kb : 128 * (kb + 1), :]
    ft_pack[0:126, 3, :] = fT[384:510, :]
    ft_pack[126:128, 3, :] = 1.0

    # ct pack: same dim map, chunk-major [128, 4*CP]
    ct_pack = np.zeros((128, 4, CP), dtype=f8)
    cT = (centers.T * CS).astype(f8)  # [D, C]
    for kb in range(3):
        ct_pack[:, kb, :C] = cT[128 * kb : 128 * (kb + 1), :]
    ct_pack[0:126, 3, :C] = cT[384:510, :]
    ct_cm = np.ascontiguousarray(
        ct_pack.reshape(128, 4, NCH, CSZ).transpose(0, 2, 1, 3).reshape(128, 4 * CP)
    )

    crs_pad = np.zeros((CP, D), dtype=bf)
    crs_pad[:C, :] = centers.astype(bf)
    fnat_bf = feat.astype(bf)
    padrow = np.full((1, CP - C), -240.0, dtype=f8)

    in_maps = []
    for i in range(NCORES):
        sl = slice(i * NPC, (i + 1) * NPC)
        in_maps.append(
            {
                "ftt": np.ascontiguousarray(ft_pack[:, :, sl]),
                "fnat": np.ascontiguousarray(fnat_bf[sl]),
                "lab": np.ascontiguousarray(label[sl]),
                "ct": ct_cm,
                "crs": crs_pad,
                "padrow": padrow,
            }
        )
    return in_maps


def combine(parts):
    nll_sum, glab_sum, s1 = np.asarray(parts, dtype=np.float64).sum(axis=0)
    centerloss = (s1 - glab_sum) / (2.0 * N)
    ddaloss = nll_sum / (2.0 * N * N)
    loss = LAMB * centerloss + GAMMA * ddaloss
    return loss, centerloss, ddaloss


def kernel(feat, label, centers):
    from concourse.bass_utils import run_bass_kernel_spmd

    in_maps = make_in_maps(feat, label, centers)
    nc = _get_nc()
    res = run_bass_kernel_spmd(nc, in_maps, core_ids=list(range(NCORES)))
    parts = [r["out"].reshape(3) for r in res.results]
    loss, centerloss, ddaloss = combine(parts)
    return (
        np.float32(loss),
        np.float32(centerloss),
        np.float32(ddaloss),
    )


# revision 19
# speedup vs baseline: 5.0199x; 1.0301x over previous
"""DDALoss Trainium2 kernel v4 (8 NeuronCores, data-parallel over batch).

Math (algebraically identical to the reference up to tiny fp8/trunc noise):
  g[n,c]     = 2*feat[n]@centers[c] - ||centers[c]||^2          (logits shifted
               by the row-constant ||feat[n]||^2, which cancels in softmax)
  lse[n]     = log(sum_c exp(g[n,c]))
  glab[n]    = g[n, label[n]]
  nll_sum    = sum_n (lse[n] - glab[n])
  S1         = sum(feat^2)
  centerloss = (S1 - sum_n glab[n]) / (2N)
  ddaloss    = nll_sum / (2N^2)
  loss       = LAMB*centerloss + GAMMA*ddaloss

v4 design (per core: 512 rows, 10240 padded classes, 20 chunks of 512):
  - csq on the PE: per 128-class tile, a 2-pass fp8-DoubleRow Gram matmul
    (ct-tile as both weights and moving) puts CS^2*csq on the psum diagonal
    (128 cols/pass vs 512 for a bias pass -> half the PE cost of the old
    K=1 bias matmuls, and no 10.5MB cnat load / DVE square pass at all).
  - diagonal extraction: DVE copy psum->bf16 with scale -FS/(2CS), bounce
    [128,512] to DRAM, read back with a stride-513 AP -> [128,4] diag.
  - bias inject: diag split hi/lo fp8 (DVE casts), written to DRAM in class
    order, then patched into partitions 126/127 of each ct chunk's k3 block.
    ft rows 126/127 of k3 are 1.0, so the second matmul pass adds
    -(FS*CS/2)*csq into psum for free (feat dims 510/511 are dropped from
    the softmax cross term only; effect ~1e-5 relative).
  - main loop: per (nt, group of 3 chunks): 2 DR passes per chunk into a
    [128,1536] psum, then one ACT exp (scale 2/(FS*CS)) with accum_out.
  - label path (indirect gather + bf16 TTR dots) and finals as before.
"""

import sys

sys.path.insert(0, "/opt/trn_rl_repo")

import numpy as np
import ml_dtypes

from contextlib import ExitStack

import concourse.bass as bass
import concourse.bacc as bacc
import concourse.tile as tile
from concourse import mybir

# Problem constants (hardcoded per harness contract)
N = 4096
D = 512
C = 10000
CP = 10240  # classes padded to 128*80
NCORES = 8
NPC = N // NCORES  # 512 rows per core
NT = NPC // 128  # 4 partition tiles per core
NCH = 20  # class chunks of 512
CSZ = 512
GRPS = [2] * 10  # ACT groups, in chunks
NGRP = len(GRPS)
GRP_OFF = [sum(GRPS[:i]) for i in range(NGRP)]
assert sum(GRPS) == NCH

LAMB = 0.01
GAMMA = 3.0

BF16 = mybir.dt.bfloat16
FP8 = mybir.dt.float8e4
F32 = mybir.dt.float32
I32 = mybir.dt.int32

# fp8 scaling: feat*FS and centers*CS on host keep e4m3 values in the normal
# range; psum holds FS*CS*cross + bias rows, ACT's exp scale of 2/(FS*CS)
# restores exp(2*cross - csq).
FS = 8.0
CS = 16.0
BSC = -FS / (2.0 * CS)  # psum-units bias = BSC * (CS^2 * csq) = -(FS*CS/2)*csq

_CACHE = {}


def _ttr(nc, out, in0, in1, accum_out, init, scale=1.0):
    """accum_out = init + sum_free(in0 * in1 * scale); out = elementwise scratch."""
    from concourse.dve_ops import TENSOR_TENSOR_REDUCE

    nc.vector._custom_dve(
        TENSOR_TENSOR_REDUCE,
        out=out,
        in0=in0,
        in1=in1,
        s0=init,
        s1=scale,
        accum_out=accum_out,
    )


def _build():
    nc = bacc.Bacc("TRN2", target_bir_lowering=False, debug=False)

    # Per-core external inputs
    ft_d = nc.dram_tensor("ftt", [128, 4, NPC], FP8, kind="ExternalInput")
    ct_d = nc.dram_tensor("ct", [128, 4 * CP], FP8, kind="ExternalInput")  # chunk-major
    fnat = nc.dram_tensor("fnat", [NPC, D], BF16, kind="ExternalInput")
    lab = nc.dram_tensor("lab", [NPC, 1], I32, kind="ExternalInput")
    crs = nc.dram_tensor("crs", [CP, D], BF16, kind="ExternalInput")  # gather rows
    padrow = nc.dram_tensor("padrow", [1, CP - C], FP8, kind="ExternalInput")
    out = nc.dram_tensor("out", [1, 3], F32, kind="ExternalOutput")
    # internal scratch
    csq2 = nc.dram_tensor("csq2_scr", [2, CP], FP8, kind="Internal")

    with tile.TileContext(nc) as tc, ExitStack() as ctx:
        const = ctx.enter_context(tc.tile_pool(name="const", bufs=1))
        small = ctx.enter_context(tc.tile_pool(name="small", bufs=2))
        gbfp = ctx.enter_context(tc.tile_pool(name="gbfp", bufs=3))
        dgp = ctx.enter_context(tc.tile_pool(name="dgp", bufs=6))
        expp = ctx.enter_context(tc.tile_pool(name="expp", bufs=2))
        scrp = ctx.enter_context(tc.tile_pool(name="scrp", bufs=2))
        ps_g = ctx.enter_context(tc.tile_pool(name="ps_g", bufs=3, space="PSUM"))
        ps_gram = ctx.enter_context(tc.tile_pool(name="ps_gram", bufs=2, space="PSUM"))

        # ---- constants / persistent tiles ----
        ones_f = const.tile([128, 1], F32)
        nc.vector.memset(ones_f, 1.0)
        ident = const.tile([128, 128], F32, tag="ident")
        from concourse.masks import make_identity

        make_identity(nc, ident)

        ft = const.tile([128, 4, NPC], FP8, tag="ft")
        nc.sync.dma_start(out=ft, in_=ft_d.ap())

        ct_all = const.tile([128, NCH, 4, CSZ], FP8, tag="ct_all")
        accg = const.tile([128, NT * NGRP], F32, tag="accg")
        cl4 = const.tile([128, NT], F32, tag="cl4")
        cq4 = const.tile([128, NT], F32, tag="cq4")
        fsq4 = const.tile([128, NT], F32, tag="fsq4")
        fin3 = const.tile([128, 3], F32, tag="fin3")

        ct_ap = ct_d.ap().rearrange("p (ch k c) -> p ch k c", k=4, c=CSZ)

        ct_dmas = {}

        def ct_dma(ci):
            ct_dmas[ci] = nc.sync.dma_start(
                out=ct_all[:, ci, :, :], in_=ct_ap[:, ci, :, :]
            )

        dhb_tiles = {}  # batch index -> [128, 16] diag tile (col = 4*(ci%4)+t)

        def chain_head(ci):
            # Gram: psum[a, 128t+b] = sum_d (CS c[d,128t+a])(CS c[d,128t+b]);
            # diag extracted by identity-masked TTR straight off PSUM.
            gp = ps_gram.tile([128, CSZ], F32, tag="gp")
            for t in range(4):
                cb = slice(128 * t, 128 * (t + 1))
                nc.tensor.matmul(
                    out=gp[:, cb],
                    lhsT=ct_all[:, ci, 0:2, cb],
                    rhs=ct_all[:, ci, 0:2, cb],
                    start=True,
                    stop=False,
                    perf_mode=mybir.MatmulPerfMode.DoubleRow,
                )
                nc.tensor.matmul(
                    out=gp[:, cb],
                    lhsT=ct_all[:, ci, 2:4, cb],
                    rhs=ct_all[:, ci, 2:4, cb],
                    start=False,
                    stop=True,
                    perf_mode=mybir.MatmulPerfMode.DoubleRow,
                )
            b = ci // 4
            if b not in dhb_tiles:
                dhb_tiles[b] = dgp.tile([128, 16], F32, tag="dhb", name=f"dhb{b}")
            dhb = dhb_tiles[b]
            for t in range(4):
                scr = gbfp.tile([128, 128], F32, tag="dscr")
                _ttr(
                    nc,
                    scr,
                    gp[:, 128 * t : 128 * (t + 1)],
                    ident,
                    dhb[:, 4 * (ci % 4) + t : 4 * (ci % 4) + t + 1],
                    0.0,
                    scale=BSC,
                )

        def tail_batch(b):
            # 4 chunks: transpose [128,16] -> [16,128]; row r=4c+t -> class
            # offset 128*r; fp8 hi/lo; contiguous DMAs; patch each chunk.
            dhb = dhb_tiles.pop(b)
            tp = ps_gram.tile([128, CSZ], F32, tag="gp")
            nc.tensor.transpose(out=tp[0:16, 0:128], in_=dhb, identity=ident)
            dtr = dgp.tile([16, 128], F32, tag="dtr")
            nc.vector.tensor_copy(dtr, tp[0:16, 0:128])
            h8 = dgp.tile([16, 128], FP8, tag="h8")
            nc.vector.tensor_copy(h8, dtr)
            hb = dgp.tile([16, 128], F32, tag="hb")
            nc.vector.tensor_copy(hb, h8)
            lo = dgp.tile([16, 128], F32, tag="lo")
            nc.vector.tensor_sub(lo, dtr, hb)
            l8 = dgp.tile([16, 128], FP8, tag="l8")
            nc.vector.tensor_copy(l8, lo)
            nc.sync.dma_start(
                out=bass.AP(tensor=csq2, offset=2048 * b, ap=[[128, 16], [1, 128]]),
                in_=h8,
            )
            nc.sync.dma_start(
                out=bass.AP(tensor=csq2, offset=CP + 2048 * b, ap=[[128, 16], [1, 128]]),
                in_=l8,
            )
            if b == 4:
                # pad classes: force hi row to -240 (-> exp(-3.75) ~ 0.02 per pad)
                nc.sync.dma_start(
                    out=bass.AP(tensor=csq2, offset=C, ap=[[0, 1], [1, CP - C]]),
                    in_=padrow.ap(),
                )
            for ci in range(4 * b, 4 * b + 4):
                nc.sync.dma_start(
                    out=ct_all[126:128, ci, 3, :],
                    in_=bass.AP(tensor=csq2, offset=512 * ci, ap=[[CP, 2], [1, 512]]),
                )

        # ---- prologue: prefetch + pipeline heads ----
        for ci in range(10):
            ct_dma(ci)
        dma_cursor = 10
        for ci in range(8):
            chain_head(ci)
        head_cursor = 8
        tail_batch(0)
        tail_cursor = 1

        def main_group(nt, g):
            w = GRPS[g] * CSZ
            gps = ps_g.tile([128, GRPS[0] * CSZ], F32, tag="gps")
            for j in range(GRPS[g]):
                ci = GRP_OFF[g] + j
                cb = slice(CSZ * j, CSZ * (j + 1))
                nc.tensor.matmul(
                    out=gps[:, cb],
                    lhsT=ft[:, 0:2, nt * 128 : (nt + 1) * 128],
                    rhs=ct_all[:, ci, 0:2, :],
                    start=True,
                    stop=False,
                    perf_mode=mybir.MatmulPerfMode.DoubleRow,
                )
                nc.tensor.matmul(
                    out=gps[:, cb],
                    lhsT=ft[:, 2:4, nt * 128 : (nt + 1) * 128],
                    rhs=ct_all[:, ci, 2:4, :],
                    start=False,
                    stop=True,
                    perf_mode=mybir.MatmulPerfMode.DoubleRow,
                )
            se = expp.tile([128, GRPS[0] * CSZ], BF16, tag="se")
            col = nt * NGRP + g
            nc.scalar.activation(
                se[:, :w],
                gps[:, :w],
                mybir.ActivationFunctionType.Exp,
                scale=2.0 / (FS * CS),
                accum_out=accg[:, col : col + 1],
            )

        # phase A: nt0/nt1 interleaved, heads/tails paced between groups
        for s in range(2 * NGRP):
            while head_cursor < min(8 + 2 * (s + 1), NCH):
                if dma_cursor < NCH:
                    ct_dma(dma_cursor)
                    dma_cursor += 1
                chain_head(head_cursor)
                head_cursor += 1
            while tail_cursor <= 4 and head_cursor >= 4 * (tail_cursor + 1):
                tail_batch(tail_cursor)
                tail_cursor += 1
            main_group(s % 2, s // 2)
        # phase B: nt2, nt3
        for nt in (2, 3):
            for g in range(NGRP):
                main_group(nt, g)

            if nt == 2:
                # ---- label path (independent; fills engine gaps late) ----
                for lt in range(NT):
                    labt = small.tile([128, 1], I32, tag="labt")
                    d1 = nc.sync.dma_start(
                        out=labt, in_=lab.ap()[lt * 128 : (lt + 1) * 128, :]
                    )
                    tile.add_dep_helper(
                        d1.ins, ct_dmas[NCH - 1].ins, True, "defer label path"
                    )
                    crows = small.tile([128, D], BF16, tag="crows")
                    nc.gpsimd.indirect_dma_start(
                        out=crows,
                        out_offset=None,
                        in_=crs.ap(),
                        in_offset=bass.IndirectOffsetOnAxis(ap=labt[:, :1], axis=0),
                    )
                    fnt = small.tile([128, D], BF16, tag="fnt")
                    d2 = nc.sync.dma_start(
                        out=fnt, in_=fnat.ap()[lt * 128 : (lt + 1) * 128, :]
                    )
                    tile.add_dep_helper(
                        d2.ins, ct_dmas[NCH - 1].ins, True, "defer label path"
                    )
                    scr1 = scrp.tile([128, D], BF16, tag="lab_scr")
                    _ttr(nc, scr1, fnt, crows, cl4[:, lt : lt + 1], 0.0)
                    scr2 = scrp.tile([128, D], BF16, tag="lab_scr")
                    _ttr(nc, scr2, crows, crows, cq4[:, lt : lt + 1], 0.0)
                    scr3 = scrp.tile([128, D], BF16, tag="lab_scr")
                    _ttr(nc, scr3, fnt, fnt, fsq4[:, lt : lt + 1], 0.0)

        # ---- finals ----
        sumexp4 = small.tile([128, NT], F32, tag="sumexp4")
        nc.vector.reduce_sum(
            sumexp4,
            accg[:, :].rearrange("p (nt g) -> p nt g", g=NGRP),
            axis=mybir.AxisListType.X,
        )
        lse4 = small.tile([128, NT], F32, tag="lse4")
        nc.scalar.activation(lse4, sumexp4, mybir.ActivationFunctionType.Ln)
        glab4 = small.tile([128, NT], F32, tag="glab4")
        nc.vector.tensor_scalar_mul(glab4, cl4, 2.0)
        nc.vector.tensor_sub(glab4, glab4, cq4)
        nld4 = small.tile([128, NT], F32, tag="nld4")
        nc.vector.tensor_sub(nld4, lse4, glab4)
        nc.vector.reduce_sum(fin3[:, 0:1], nld4, axis=mybir.AxisListType.X)
        nc.vector.reduce_sum(fin3[:, 1:2], glab4, axis=mybir.AxisListType.X)
        nc.vector.reduce_sum(fin3[:, 2:3], fsq4, axis=mybir.AxisListType.X)
        fin_ps = ps_gram.tile([128, CSZ], F32, tag="gp")
        nc.tensor.matmul(
            out=fin_ps[0:1, 0:3], lhsT=ones_f, rhs=fin3, start=True, stop=True
        )
        out_sb = small.tile([1, 3], F32, tag="out_sb")
        nc.scalar.copy(out_sb, fin_ps[0:1, 0:3])
        nc.sync.dma_start(out=out.ap(), in_=out_sb)

    nc.compile()
    return nc


def _get_nc():
    if "nc" not in _CACHE:
        _CACHE["nc"] = _build()
    return _CACHE["nc"]


def make_in_maps(feat, label, centers):
    feat = np.ascontiguousarray(np.asarray(feat, dtype=np.float32))
    centers = np.ascontiguousarray(np.asarray(centers, dtype=np.float32))
    label = np.ascontiguousarray(np.asarray(label).astype(np.int32).reshape(N, 1))

    bf = ml_dtypes.bfloat16
    f8 = ml_dtypes.float8_e4m3

    # ft pack: [128, 4, N]; k3 = dims 384..509 + two ones rows
    ft_pack = np.zeros((128, 4, N), dtype=f8)
    fT = (feat.T * FS).astype(f8)  # [D, N]
    for kb in range(3):
        ft_pack[:, kb, :] = fT[128 * kb : 128 * (kb + 1), :]
    ft_pack[0:126, 3, :] = fT[384:510, :]
    ft_pack[126:128, 3, :] = 1.0

    # ct pack: same dim map, chunk-major [128, 4*CP]
    ct_pack = np.zeros((128, 4, CP), dtype=f8)
    cT = (centers.T * CS).astype(f8)  # [D, C]
    for kb in range(3):
        ct_pack[:, kb, :C] = cT[128 * kb : 128 * (kb + 1), :]
    ct_pack[0:126, 3, :C] = cT[384:510, :]
    ct_cm = np.ascontiguousarray(
        ct_pack.reshape(128, 4, NCH, CSZ).transpose(0, 2, 1, 3).reshape(128, 4 * CP)
    )

    crs_pad = np.zeros((CP, D), dtype=bf)
    crs_pad[:C, :] = centers.astype(bf)
    fnat_bf = feat.astype(bf)
    padrow = np.full((1, CP - C), -240.0, dtype=f8)

    in_maps = []
    for i in range(NCORES):
        sl = slice(i * NPC, (i + 1) * NPC)
        in_maps.append(
            {
                "ftt": np.ascontiguousarray(ft_pack[:, :, sl]),
                "fnat": np.ascontiguousarray(fnat_bf[sl]),
                "lab": np.ascontiguousarray(label[sl]),
                "ct": ct_cm,
                "crs": crs_pad,
                "padrow": padrow,
            }
        )
    return in_maps


def combine(parts):
    nll_sum, glab_sum, s1 = np.asarray(parts, dtype=np.float64).sum(axis=0)
    centerloss = (s1 - glab_sum) / (2.0 * N)
    ddaloss = nll_sum / (2.0 * N * N)
    loss = LAMB * centerloss + GAMMA * ddaloss
    return loss, centerloss, ddaloss


def kernel(feat, label, centers):
    from concourse.bass_utils import run_bass_kernel_spmd

    in_maps = make_in_maps(feat, label, centers)
    nc = _get_nc()
    res = run_bass_kernel_spmd(nc, in_maps, core_ids=list(range(NCORES)))
    parts = [r["out"].reshape(3) for r in res.results]
    loss, centerloss, ddaloss = combine(parts)
    return (
        np.float32(loss),
        np.float32(centerloss),
        np.float32(ddaloss),
    )
